# revision 1
# baseline (speedup 1.0000x reference)
"""Trainium2 Bass kernel for the tied-embedding LSTM LM loss.

Structure (per the vocab-tensor-parallel sharding):
  Phase A: XW = emb[x] @ W_ih  for all (t,b) pairs        -- replicated
  Phase B: 128-step LSTM recurrence (g = XW_t + h_t @ W_hh) -- replicated
  Phase C: OUT.T = Wr @ H2.T ; logits = OUT @ emb_shard.T  -- vocab-sharded
           per-row sum(exp(logit)) partials + target-logit dots
  Host:    combine 8 sumexp partials, log-sum-exp, mask, reduce to scalar.

All matmuls run in bf16 (fp32 PSUM accumulation); LSTM cell state is fp32.
"""

import numpy as np
import ml_dtypes

import concourse.bass as bass
import concourse.bacc as bacc
import concourse.mybir as mybir
import concourse.tile as tile
from concourse.bass_utils import run_bass_kernel_spmd

FP32 = mybir.dt.float32
BF16 = mybir.dt.bfloat16
AF = mybir.ActivationFunctionType
ALU = mybir.AluOpType

V, E, H = 32000, 1024, 1024
T1, B = 129, 64
TX = T1 - 1               # 128 recurrence steps
R = TX * B                # 8192 (t,b) rows
NC = 8                    # cores
VS = V // NC              # 4000 vocab shard
KC = E // 128             # 8 contraction chunks
MC = R // 128             # 64 row chunks
NBLK = 16                 # 512-wide OUT.T column blocks
BW = R // NBLK            # 512


def build_program():
    nc = bacc.Bacc("TRN2", target_bir_lowering=False)

    # ---- inputs (per-core layouts prepared on host) ----
    xt = nc.dram_tensor("xt", [MC, 128, KC, 128], BF16, kind="ExternalInput")
    wih = nc.dram_tensor("wih", [128, KC, 4 * H], BF16, kind="ExternalInput")
    whh = nc.dram_tensor("whh", [128, KC, 4 * H], BF16, kind="ExternalInput")
    wrt = nc.dram_tensor("wrt", [128, KC, E], BF16, kind="ExternalInput")
    embt = nc.dram_tensor("embt", [128, KC, VS], BF16, kind="ExternalInput")
    eyt = nc.dram_tensor("eyt", [128, KC, R], BF16, kind="ExternalInput")
    ident = nc.dram_tensor("ident", [64, 64], BF16, kind="ExternalInput")
    ones128 = nc.dram_tensor("ones128", [128, 1], BF16, kind="ExternalInput")

    # ---- outputs ----
    s_out = nc.dram_tensor("s_out", [128, MC], FP32, kind="ExternalOutput")
    t_out = nc.dram_tensor("t_out", [NBLK, BW], FP32, kind="ExternalOutput")

    # ---- DRAM scratch ----
    xw_d = nc.dram_tensor("xw_d", [MC, 128, 4 * H], BF16, kind="Internal")
    outt_d = nc.dram_tensor("outt_d", [128, KC, R], BF16, kind="Internal")

    with tile.TileContext(nc) as tc:
        with (
            tc.tile_pool(name="psum", bufs=2, space="PSUM") as pp,
            tc.tile_pool(name="small", bufs=1) as smp,
        ):
            id_sb = smp.tile([64, 64], BF16, tag="id")
            nc.sync.dma_start(id_sb[:], ident[:])
            ones_sb = smp.tile([128, 1], BF16, tag="ones")
            nc.sync.dma_start(ones_sb[:], ones128[:])
            s_sb = smp.tile([128, MC], FP32, tag="s")

            # ================= Phase A: XW = X @ W_ih =================
            with (
                tc.tile_pool(name="wih_p", bufs=1) as wih_p,
                tc.tile_pool(name="a_io", bufs=3) as a_io,
            ):
                wih_sb = wih_p.tile([128, KC, 4 * H], BF16, tag="w")
                nc.sync.dma_start(wih_sb[:], wih[:])
                for mc in range(MC):
                    xt_sb = a_io.tile([128, KC, 128], BF16, tag="xt")
                    nc.sync.dma_start(xt_sb[:], xt[mc])
                    for hf in range(2):
                        ps = pp.tile([128, 2048], FP32, tag="ps")
                        for k in range(KC):
                            for nn in range(4):
                                nc.tensor.matmul(
                                    ps[:, nn * 512:(nn + 1) * 512],
                                    lhsT=xt_sb[:, k, :],
                                    rhs=wih_sb[:, k, hf * 2048 + nn * 512:
                                               hf * 2048 + (nn + 1) * 512],
                                    start=(k == 0), stop=(k == KC - 1),
                                )
                        xw_sb = a_io.tile([128, 2048], BF16, tag="xw")
                        nc.any.tensor_copy(xw_sb[:], ps[:])
                        nc.sync.dma_start(
                            xw_d[mc, :, hf * 2048:(hf + 1) * 2048], xw_sb[:])

            # ================= Phase B: LSTM recurrence =================
            with (
                tc.tile_pool(name="whh_p", bufs=1) as whh_p,
                tc.tile_pool(name="b_io", bufs=2) as b_io,
                tc.tile_pool(name="b_st", bufs=2) as b_st,
            ):
                whh_sb = whh_p.tile([128, KC, 4 * H], BF16, tag="w")
                nc.sync.dma_start(whh_sb[:], whh[:])
                wrt_sb = whh_p.tile([128, KC, E], BF16, tag="wrt")
                nc.sync.dma_start(wrt_sb[:], wrt[:])

                ht_sb = b_st.tile([128, KC, 64], BF16, tag="ht")
                ct_sb = b_st.tile([64, H], FP32, tag="ct")
                nc.any.memset(ht_sb[:], 0.0)
                nc.any.memset(ct_sb[:], 0.0)

                for t in range(TX):
                    xwb = b_io.tile([64, 4 * H], BF16, tag="xwb")
                    nc.sync.dma_start(
                        xwb[:], xw_d[t // 2, (t % 2) * 64:(t % 2) * 64 + 64, :])

                    ghalf = []
                    for hf in range(2):
                        g = pp.tile([64, 2048], FP32, tag="ps")
                        for nn in range(4):
                            nc.tensor.matmul(
                                g[:, nn * 512:(nn + 1) * 512],
                                lhsT=id_sb[:],
                                rhs=xwb[:, hf * 2048 + nn * 512:
                                        hf * 2048 + (nn + 1) * 512],
                                start=True, stop=False,
                            )
                        for k in range(KC):
                            for nn in range(4):
                                nc.tensor.matmul(
                                    g[:, nn * 512:(nn + 1) * 512],
                                    lhsT=ht_sb[:, k, :],
                                    rhs=whh_sb[:, k, hf * 2048 + nn * 512:
                                               hf * 2048 + (nn + 1) * 512],
                                    start=False, stop=(k == KC - 1),
                                )
                        ghalf.append(g)

                    gates = b_io.tile([64, 4 * H], FP32, tag="gates")
                    # layout: [i | f] from half0, [gg | o] from half1
                    nc.scalar.activation(gates[:, 0:2048], ghalf[0][:, 0:2048],
                                         AF.Sigmoid)
                    nc.scalar.activation(gates[:, 2048:3072], ghalf[1][:, 0:1024],
                                         AF.Tanh)
                    nc.scalar.activation(gates[:, 3072:4096], ghalf[1][:, 1024:2048],
                                         AF.Sigmoid)

                    t1 = b_io.tile([64, H], FP32, tag="t1")
                    nc.vector.tensor_tensor(t1[:], gates[:, 0:1024],
                                            gates[:, 2048:3072], op=ALU.mult)
                    t2 = b_io.tile([64, H], FP32, tag="t2")
                    nc.vector.tensor_tensor(t2[:], gates[:, 1024:2048],
                                            ct_sb[:], op=ALU.mult)
                    cn = b_st.tile([64, H], FP32, tag="ct")
                    nc.vector.tensor_tensor(cn[:], t1[:], t2[:], op=ALU.add)
                    tn = b_io.tile([64, H], FP32, tag="tn")
                    nc.scalar.activation(tn[:], cn[:], AF.Tanh)
                    hn = b_io.tile([64, H], BF16, tag="hn")
                    nc.vector.tensor_tensor(hn[:], gates[:, 3072:4096], tn[:],
                                            op=ALU.mult)
                    ct_sb = cn

                    trp = pp.tile([128, 512], BF16, tag="ps")
                    for k in range(KC):
                        nc.tensor.transpose(
                            trp[:, k * 64:(k + 1) * 64],
                            hn[:, k * 128:(k + 1) * 128], id_sb[:])
                    ht_sb = b_st.tile([128, KC, 64], BF16, tag="ht")
                    nc.any.tensor_copy(ht_sb[:], trp[:])

                    # readout OUT.T columns for this step -- fills the PE
                    # idle tail (keeps HAM warm) and removes phase-C1
                    rop = pp.tile([128, 512], FP32, tag="ps")
                    for m in range(KC):
                        for k in range(KC):
                            nc.tensor.matmul(
                                rop[:, m * 64:(m + 1) * 64],
                                lhsT=wrt_sb[:, k, m * 128:(m + 1) * 128],
                                rhs=ht_sb[:, k, :],
                                start=(k == 0), stop=(k == KC - 1))
                    ro_sb = b_io.tile([128, KC, 64], BF16, tag="ro")
                    nc.any.tensor_copy(ro_sb[:], rop[:])
                    nc.sync.dma_start(outt_d[:, :, t * 64:(t + 1) * 64], ro_sb[:])

            # ================= Phase C: readout + decoder =================
            with (
                tc.tile_pool(name="c_w", bufs=1) as c_w,
                tc.tile_pool(name="c_io", bufs=2) as c_io,
                tc.tile_pool(name="c_sc", bufs=2) as c_sc,
            ):
                embt_sb = c_w.tile([128, KC, VS], BF16, tag="embt")
                nc.sync.dma_start(embt_sb[:], embt[:])

                for nb in range(NBLK):
                    outt = c_io.tile([128, KC, BW], BF16, tag="outt")
                    nc.sync.dma_start(outt[:], outt_d[:, :, nb * BW:(nb + 1) * BW])

                    # decoder: 4 row-chunks of 128 rows each
                    for mm in range(4):
                        gmc = nb * 4 + mm
                        sacc = c_sc.tile([128, 2], FP32, tag="sacc")
                        for hf in range(2):
                            ps2 = pp.tile([128, 2000], FP32, tag="ps")
                            for k in range(KC):
                                for nn in range(4):
                                    nc.tensor.matmul(
                                        ps2[:, nn * 500:(nn + 1) * 500],
                                        lhsT=outt[:, k, mm * 128:(mm + 1) * 128],
                                        rhs=embt_sb[:, k, hf * 2000 + nn * 500:
                                                    hf * 2000 + (nn + 1) * 500],
                                        start=(k == 0), stop=(k == KC - 1))
                            esc = c_sc.tile([128, 2000], BF16, tag="esc")
                            nc.scalar.activation(esc[:], ps2[:], AF.Exp,
                                                 accum_out=sacc[:, hf:hf + 1])
                        nc.vector.tensor_tensor(s_sb[:, gmc:gmc + 1],
                                                sacc[:, 0:1], sacc[:, 1:2],
                                                op=ALU.add)

                    # target-logit dots for these 512 rows (all cores redundant)
                    eyb = c_io.tile([128, KC, BW], BF16, tag="eyb")
                    nc.sync.dma_start(eyb[:], eyt[:, :, nb * BW:(nb + 1) * BW])
                    prod = c_io.tile([128, KC, BW], BF16, tag="prod")
                    nc.vector.tensor_tensor(prod[:], outt[:], eyb[:], op=ALU.mult)
                    tps = pp.tile([1, BW], FP32, tag="ps")
                    for k in range(KC):
                        nc.tensor.matmul(tps[:], lhsT=ones_sb[:], rhs=prod[:, k, :],
                                         start=(k == 0), stop=(k == KC - 1))
                    tsb = c_sc.tile([1, BW], FP32, tag="tsb")
                    nc.any.tensor_copy(tsb[:], tps[:])
                    nc.sync.dma_start(t_out[nb:nb + 1, :], tsb[:])

            nc.sync.dma_start(s_out[:], s_sb[:])

    nc.compile()
    return nc


_PROGRAM = None


def _get_program():
    global _PROGRAM
    if _PROGRAM is None:
        _PROGRAM = build_program()
    return _PROGRAM


def _prep_inputs(data, mask, emb, W_ih, W_hh, b, Wr, br, bd):
    assert not np.any(b) and not np.any(br), "nonzero LSTM/readout bias unsupported"
    bf = ml_dtypes.bfloat16
    x = np.ascontiguousarray(data[:-1]).astype(np.int64).reshape(-1)
    y = np.ascontiguousarray(data[1:]).astype(np.int64).reshape(-1)

    X = emb[x]                                    # [R, E] fp32
    # xt[mc, p, k, m] = X[mc*128 + m, k*128 + p]
    xt = np.ascontiguousarray(
        X.reshape(MC, 128, KC, 128).transpose(0, 3, 2, 1)).astype(bf)
    wih = np.ascontiguousarray(
        W_ih.reshape(KC, 128, 4 * H).transpose(1, 0, 2)).astype(bf)
    whh = np.ascontiguousarray(
        W_hh.reshape(KC, 128, 4 * H).transpose(1, 0, 2)).astype(bf)
    # wrt[p, k, e] = Wr[e, k*128 + p]
    wrt = np.ascontiguousarray(
        Wr.T.reshape(KC, 128, E).transpose(1, 0, 2)).astype(bf)
    EY = emb[y]                                   # [R, E]
    eyt = np.ascontiguousarray(
        EY.T.reshape(KC, 128, R).transpose(1, 0, 2)).astype(bf)
    ident = np.eye(64, dtype=bf)
    ones = np.ones((128, 1), dtype=bf)

    in_maps = []
    for j in range(NC):
        shard = emb[j * VS:(j + 1) * VS]          # [VS, E]
        embt = np.ascontiguousarray(
            shard.T.reshape(KC, 128, VS).transpose(1, 0, 2)).astype(bf)
        in_maps.append({
            "xt": xt, "wih": wih, "whh": whh, "wrt": wrt,
            "embt": embt, "eyt": eyt, "ident": ident, "ones128": ones,
        })
    return in_maps, y


def _combine(results, y, mask, bd):
    S = np.zeros(R, np.float64)
    for j in range(NC):
        # s_out[p, mc] -> row mc*128 + p
        S += results[j]["s_out"].T.reshape(-1).astype(np.float64)
    Tt = results[0]["t_out"].reshape(-1).astype(np.float64) + bd[y]
    m = mask[1:].reshape(-1).astype(np.float64)
    nll = np.log(S) - Tt
    loss = (nll * m).sum() / (B * B)
    return np.float32(loss)


def _run(in_maps, **kw):
    nc = _get_program()
    return run_bass_kernel_spmd(nc, in_maps, core_ids=list(range(NC)), **kw)


def kernel(data, mask, emb, W_ih, W_hh, b, Wr, br, bd):
    data = np.asarray(data)
    mask = np.asarray(mask).astype(np.float32)
    emb = np.asarray(emb).astype(np.float32)
    args = dict(data=data, mask=mask, emb=emb,
                W_ih=np.asarray(W_ih, np.float32),
                W_hh=np.asarray(W_hh, np.float32),
                b=np.asarray(b, np.float32), Wr=np.asarray(Wr, np.float32),
                br=np.asarray(br, np.float32), bd=np.asarray(bd, np.float32))
    in_maps, y = _prep_inputs(**args)
    res = _run(in_maps)
    return _combine(res.results, y, mask, np.asarray(bd, np.float64))



# revision 3
# speedup vs baseline: 1.1888x; 1.1888x over previous
"""Trainium2 Bass kernel for the tied-embedding LSTM LM loss — v3.

v3 = v2 + int8 wire format for the three big data tensors (embt, xt,
eyt), all quantized with one global scale s8 = max|emb|/127.  The
integer values flow through the bf16 matmuls exactly; s8 is folded into
the XW PSUM->SBUF copy (activation scale), the decoder Exp scale, and a
host-side multiply for the target dots.  Upload drops ~48 MB.


v1 replicated every input across the 8 cores (~58 MB/core, ~464 MB per
call over the axon tunnel) — host->device transfer dominated the wall
clock.  v2 uploads each unique byte once (sharded across cores,
~14.3 MB/core) and reassembles on device with collectives:

  upload (per core j):
    xtj    [8,128,KC,128]  X rows [1024j,1024j+1024)   (2 MB)
    wihj   [16,KC,4096]    W_ih partition rows 16j..   (1 MB)
    whhj   [16,KC,4096]    W_hh partition rows 16j..   (1 MB)
    wrtj   [16,KC,1024]    Wr.T partition rows 16j..   (0.25 MB)
    embtj  [128,KC,4000]   emb.T vocab shard           (8 MB)
    eytj   [128,KC,1024]   emb[y].T/8 row shard        (2 MB)

  device:
    AllGather W_ih, W_hh, Wr.T
    Phase A (row-sharded): XWj = Xj @ W_ih ; AllGather XW
    Phase B (replicated): 128-step LSTM recurrence; emits OUT.T into a
      row-block-outer DRAM layout outt2[rb, p, k, rl]
    ReduceScatter(outt2) -> or2 = 8 * OUT.T[:, rows_j]  (fixed address,
      avoids core-dependent offsets; the 8x is folded into eytj)
    Phase C: target-logit dots for local rows; vocab-sharded decoder
      sum(exp(logit)) partials
  host: combine partials -> scalar loss.
"""

import numpy as np
import ml_dtypes

import jax
jax.config.update("jax_compilation_cache_dir", "/tmp/jax_neff_cache")
jax.config.update("jax_persistent_cache_min_entry_size_bytes", -1)
jax.config.update("jax_persistent_cache_min_compile_time_secs", 0.0)

import concourse.bass as bass
import concourse.bacc as bacc
import concourse.mybir as mybir
import concourse.tile as tile
from concourse.bass_utils import run_bass_kernel_spmd

FP32 = mybir.dt.float32
BF16 = mybir.dt.bfloat16
I8 = mybir.dt.int8
AF = mybir.ActivationFunctionType
ALU = mybir.AluOpType

V, E, H = 32000, 1024, 1024
T1, B = 129, 64
TX = T1 - 1               # 128 recurrence steps
R = TX * B                # 8192 (t,b) rows
NC = 8                    # cores
VS = V // NC              # 4000 vocab shard
KC = E // 128             # 8 contraction chunks
MC = R // 128             # 64 global row chunks
MCL = MC // NC            # 8 local row chunks
RL = R // NC              # 1024 local rows
NBLK = 16                 # 512-wide OUT.T column blocks
BW = R // NBLK            # 512
RG = [list(range(NC))]


def build_program():
    nc = bacc.Bacc("TRN2", target_bir_lowering=False, num_devices=NC)

    # ---- sharded inputs (per-core layouts prepared on host) ----
    xtj = nc.dram_tensor("xtj", [MCL, 128, KC, 128], I8, kind="ExternalInput")
    wihj = nc.dram_tensor("wihj", [16, KC, 4 * H], I8, kind="ExternalInput")
    whhj = nc.dram_tensor("whhj", [16, KC, 4 * H], BF16, kind="ExternalInput")
    wrtj = nc.dram_tensor("wrtj", [16, KC, E], I8, kind="ExternalInput")
    embtj = nc.dram_tensor("embtj", [128, KC, VS], I8, kind="ExternalInput")
    eytj = nc.dram_tensor("eytj", [128, KC, RL], I8, kind="ExternalInput")
    ident = nc.dram_tensor("ident", [64, 64], BF16, kind="ExternalInput")
    ones128 = nc.dram_tensor("ones128", [128, 1], BF16, kind="ExternalInput")
    scl = nc.dram_tensor("scl", [128, 2], FP32, kind="ExternalInput")

    # ---- outputs ----
    s_out = nc.dram_tensor("s_out", [128, MC], FP32, kind="ExternalOutput")
    t_out = nc.dram_tensor("t_out", [1, RL], FP32, kind="ExternalOutput")

    # ---- DRAM scratch ----
    wih_b = nc.dram_tensor("wih_b", [16, KC, 4 * H], I8, kind="Internal")
    whh_b = nc.dram_tensor("whh_b", [16, KC, 4 * H], BF16, kind="Internal")
    wrt_b = nc.dram_tensor("wrt_b", [16, KC, E], I8, kind="Internal")
    wih_g = nc.dram_tensor("wih_g", [128, KC, 4 * H], I8, kind="Internal",
                           addr_space="Shared")
    whh_g = nc.dram_tensor("whh_g", [128, KC, 4 * H], BF16, kind="Internal",
                           addr_space="Shared")
    wrt_g = nc.dram_tensor("wrt_g", [128, KC, E], I8, kind="Internal",
                           addr_space="Shared")
    xwj_d = nc.dram_tensor("xwj_d", [MCL, 128, 4 * H], BF16, kind="Internal")
    xw_g = nc.dram_tensor("xw_g", [MC, 128, 4 * H], BF16, kind="Internal",
                          addr_space="Shared")
    # OUT.T in row-block-outer layout: outt2[rb, p, k, rl] =
    #   OUT.T[k*128+p, rb*1024+rl]
    outt2_d = nc.dram_tensor("outt2_d", [NC, 128, KC, RL], BF16, kind="Internal")
    or2_d = nc.dram_tensor("or2_d", [128, KC, RL], BF16, kind="Internal")

    with tile.TileContext(nc) as tc:
        with (
            tc.tile_pool(name="psum", bufs=2, space="PSUM") as pp,
            tc.tile_pool(name="small", bufs=1) as smp,
        ):
            id_sb = smp.tile([64, 64], BF16, tag="id")
            nc.sync.dma_start(id_sb[:], ident[:])
            ones_sb = smp.tile([128, 1], BF16, tag="ones")
            nc.sync.dma_start(ones_sb[:], ones128[:])
            scl_sb = smp.tile([128, 2], FP32, tag="scl")
            nc.sync.dma_start(scl_sb[:], scl[:])
            s_sb = smp.tile([128, MC], FP32, tag="s")

            # ---- bounce sharded weights into internal DRAM, then gather ----
            nc.gpsimd.dma_start(wih_b[:], wihj[:])
            nc.gpsimd.collective_compute(
                "AllGather", ALU.bypass, replica_groups=RG,
                ins=[wih_b[:]], outs=[wih_g[:]])
            nc.gpsimd.dma_start(whh_b[:], whhj[:])
            nc.gpsimd.collective_compute(
                "AllGather", ALU.bypass, replica_groups=RG,
                ins=[whh_b[:]], outs=[whh_g[:]])
            nc.gpsimd.dma_start(wrt_b[:], wrtj[:])
            nc.gpsimd.collective_compute(
                "AllGather", ALU.bypass, replica_groups=RG,
                ins=[wrt_b[:]], outs=[wrt_g[:]])

            # ================= Phase A: XWj = Xj @ W_ih (row-sharded) =======
            with (
                tc.tile_pool(name="wih_p", bufs=1) as wih_p,
                tc.tile_pool(name="a_io", bufs=3) as a_io,
            ):
                wih_sb = wih_p.tile([128, KC, 4 * H], BF16, tag="w")
                with tc.tile_pool(name="astage", bufs=1) as ast_p:
                    wih_q = ast_p.tile([128, KC, 4 * H], I8, tag="wq")
                    nc.sync.dma_start(wih_q[:], wih_g[:])
                    nc.any.tensor_copy(wih_sb[:], wih_q[:])
                for mc in range(MCL):
                    xt_q = a_io.tile([128, KC, 128], I8, tag="xtq")
                    nc.sync.dma_start(xt_q[:], xtj[mc])
                    xt_sb = a_io.tile([128, KC, 128], BF16, tag="xt")
                    nc.any.tensor_copy(xt_sb[:], xt_q[:])
                    for hf in range(2):
                        ps = pp.tile([128, 2048], FP32, tag="ps")
                        for k in range(KC):
                            for nn in range(4):
                                nc.tensor.matmul(
                                    ps[:, nn * 512:(nn + 1) * 512],
                                    lhsT=xt_sb[:, k, :],
                                    rhs=wih_sb[:, k, hf * 2048 + nn * 512:
                                               hf * 2048 + (nn + 1) * 512],
                                    start=(k == 0), stop=(k == KC - 1),
                                )
                        xw_sb = a_io.tile([128, 2048], BF16, tag="xw")
                        # dequant: XW = s8 * (qX @ W_ih)
                        nc.scalar.activation(xw_sb[:], ps[:], AF.Copy,
                                             scale=scl_sb[:, 0:1])
                        nc.sync.dma_start(
                            xwj_d[mc, :, hf * 2048:(hf + 1) * 2048], xw_sb[:])

            nc.gpsimd.collective_compute(
                "AllGather", ALU.bypass, replica_groups=RG,
                ins=[xwj_d[:]], outs=[xw_g[:]])

            # ================= Phase B: LSTM recurrence (replicated) ========
            with (
                tc.tile_pool(name="whh_p", bufs=1) as whh_p,
                tc.tile_pool(name="b_io", bufs=2) as b_io,
                tc.tile_pool(name="b_st", bufs=2) as b_st,
            ):
                whh_sb = whh_p.tile([128, KC, 4 * H], BF16, tag="w")
                nc.sync.dma_start(whh_sb[:], whh_g[:])
                wrt_sb = whh_p.tile([128, KC, E], BF16, tag="wrt")
                with tc.tile_pool(name="wstage", bufs=1) as wst:
                    wrt_q = wst.tile([128, KC, E], I8, tag="wrtq")
                    nc.sync.dma_start(wrt_q[:], wrt_g[:])
                    nc.any.tensor_copy(wrt_sb[:], wrt_q[:])

                ht_sb = b_st.tile([128, KC, 64], BF16, tag="ht")
                ct_sb = b_st.tile([64, H], FP32, tag="ct")
                nc.any.memset(ht_sb[:], 0.0)
                nc.any.memset(ct_sb[:], 0.0)

                for t in range(TX):
                    xwb = b_io.tile([64, 4 * H], BF16, tag="xwb")
                    nc.sync.dma_start(
                        xwb[:], xw_g[t // 2, (t % 2) * 64:(t % 2) * 64 + 64, :])

                    ghalf = []
                    for hf in range(2):
                        g = pp.tile([64, 2048], FP32, tag="ps")
                        for nn in range(4):
                            nc.tensor.matmul(
                                g[:, nn * 512:(nn + 1) * 512],
                                lhsT=id_sb[:],
                                rhs=xwb[:, hf * 2048 + nn * 512:
                                        hf * 2048 + (nn + 1) * 512],
                                start=True, stop=False,
                            )
                        for k in range(KC):
                            for nn in range(4):
                                nc.tensor.matmul(
                                    g[:, nn * 512:(nn + 1) * 512],
                                    lhsT=ht_sb[:, k, :],
                                    rhs=whh_sb[:, k, hf * 2048 + nn * 512:
                                               hf * 2048 + (nn + 1) * 512],
                                    start=False, stop=(k == KC - 1),
                                )
                        ghalf.append(g)

                    gates = b_io.tile([64, 4 * H], FP32, tag="gates")
                    # layout: [i | f] from half0, [gg | o] from half1
                    nc.scalar.activation(gates[:, 0:2048], ghalf[0][:, 0:2048],
                                         AF.Sigmoid)
                    nc.scalar.activation(gates[:, 2048:3072], ghalf[1][:, 0:1024],
                                         AF.Tanh)
                    nc.scalar.activation(gates[:, 3072:4096], ghalf[1][:, 1024:2048],
                                         AF.Sigmoid)

                    t1 = b_io.tile([64, H], FP32, tag="t1")
                    nc.vector.tensor_tensor(t1[:], gates[:, 0:1024],
                                            gates[:, 2048:3072], op=ALU.mult)
                    t2 = b_io.tile([64, H], FP32, tag="t2")
                    nc.vector.tensor_tensor(t2[:], gates[:, 1024:2048],
                                            ct_sb[:], op=ALU.mult)
                    cn = b_st.tile([64, H], FP32, tag="ct")
                    nc.vector.tensor_tensor(cn[:], t1[:], t2[:], op=ALU.add)
                    tn = b_io.tile([64, H], FP32, tag="tn")
                    nc.scalar.activation(tn[:], cn[:], AF.Tanh)
                    hn = b_io.tile([64, H], BF16, tag="hn")
                    nc.vector.tensor_tensor(hn[:], gates[:, 3072:4096], tn[:],
                                            op=ALU.mult)
                    ct_sb = cn

                    trp = pp.tile([128, 512], BF16, tag="ps")
                    for k in range(KC):
                        nc.tensor.transpose(
                            trp[:, k * 64:(k + 1) * 64],
                            hn[:, k * 128:(k + 1) * 128], id_sb[:])
                    ht_sb = b_st.tile([128, KC, 64], BF16, tag="ht")
                    nc.any.tensor_copy(ht_sb[:], trp[:])

                    # readout OUT.T columns for this step
                    rop = pp.tile([128, 512], FP32, tag="ps")
                    for m in range(KC):
                        for k in range(KC):
                            nc.tensor.matmul(
                                rop[:, m * 64:(m + 1) * 64],
                                lhsT=wrt_sb[:, k, m * 128:(m + 1) * 128],
                                rhs=ht_sb[:, k, :],
                                start=(k == 0), stop=(k == KC - 1))
                    ro_sb = b_io.tile([128, KC, 64], BF16, tag="ro")
                    nc.any.tensor_copy(ro_sb[:], rop[:])
                    # step t covers rows t*64..t*64+63 ->
                    # rb = t//16, rl = (t%16)*64
                    nc.sync.dma_start(
                        outt2_d[t // 16, :, :, (t % 16) * 64:(t % 16) * 64 + 64],
                        ro_sb[:])

            # identical outt2 on every core -> rank j keeps 8*chunk_j
            nc.gpsimd.collective_compute(
                "ReduceScatter", ALU.add, replica_groups=RG,
                ins=[outt2_d[:]], outs=[or2_d[:]])

            # ================= Phase C: readout + decoder =================
            with (
                tc.tile_pool(name="c_w", bufs=1) as c_w,
                tc.tile_pool(name="c_io", bufs=2) as c_io,
                tc.tile_pool(name="c_sc", bufs=2) as c_sc,
            ):
                embt_sb = c_w.tile([128, KC, VS], BF16, tag="embt")
                for hf in range(2):
                    embq = c_io.tile([128, KC, 2000], I8, tag="embq")
                    nc.sync.dma_start(embq[:], embtj[:, :, hf * 2000:(hf + 1) * 2000])
                    nc.any.tensor_copy(embt_sb[:, :, hf * 2000:(hf + 1) * 2000],
                                       embq[:])

                # target-logit dots for this core's 1024 rows (note eytj is
                # pre-scaled by 1/8 to cancel the ReduceScatter over 8
                # identical copies)
                or2_sb = c_w.tile([128, KC, RL], BF16, tag="or2")
                nc.sync.dma_start(or2_sb[:], or2_d[:])
                ey_q = c_io.tile([128, KC, RL], I8, tag="eyq")
                nc.sync.dma_start(ey_q[:], eytj[:])
                ey_sb = c_w.tile([128, KC, RL], BF16, tag="ey")
                nc.any.tensor_copy(ey_sb[:], ey_q[:])
                prod = c_w.tile([128, KC, RL], BF16, tag="prod")
                nc.vector.tensor_tensor(prod[:], or2_sb[:], ey_sb[:], op=ALU.mult)
                tsb = c_sc.tile([1, RL], FP32, tag="tsb")
                for half in range(2):
                    tps = pp.tile([1, 512], FP32, tag="ps")
                    for k in range(KC):
                        nc.tensor.matmul(
                            tps[:], lhsT=ones_sb[:],
                            rhs=prod[:, k, half * 512:(half + 1) * 512],
                            start=(k == 0), stop=(k == KC - 1))
                    nc.any.tensor_copy(tsb[:, half * 512:(half + 1) * 512], tps[:])
                nc.sync.dma_start(t_out[:], tsb[:])

                # decoder sum(exp(logit)) partials over the vocab shard
                for nb in range(NBLK):
                    outt = c_io.tile([128, KC, BW], BF16, tag="outt")
                    nc.sync.dma_start(
                        outt[:],
                        outt2_d[nb // 2, :, :,
                                (nb % 2) * BW:(nb % 2) * BW + BW])

                    for mm in range(4):
                        gmc = nb * 4 + mm
                        sacc = c_sc.tile([128, 2], FP32, tag="sacc")
                        for hf in range(2):
                            ps2 = pp.tile([128, 2000], FP32, tag="ps")
                            for k in range(KC):
                                for nn in range(4):
                                    nc.tensor.matmul(
                                        ps2[:, nn * 500:(nn + 1) * 500],
                                        lhsT=outt[:, k, mm * 128:(mm + 1) * 128],
                                        rhs=embt_sb[:, k, hf * 2000 + nn * 500:
                                                    hf * 2000 + (nn + 1) * 500],
                                        start=(k == 0), stop=(k == KC - 1))
                            esc = c_sc.tile([128, 2000], BF16, tag="esc")
                            # logit = s8 * (out . qemb)
                            nc.scalar.activation(esc[:], ps2[:], AF.Exp,
                                                 scale=scl_sb[:, 1:2],
                                                 accum_out=sacc[:, hf:hf + 1])
                        nc.vector.tensor_tensor(s_sb[:, gmc:gmc + 1],
                                                sacc[:, 0:1], sacc[:, 1:2],
                                                op=ALU.add)

            nc.sync.dma_start(s_out[:], s_sb[:])

    nc.compile()
    return nc


_PROGRAM = None


def _get_program():
    global _PROGRAM
    if _PROGRAM is None:
        _PROGRAM = build_program()
    return _PROGRAM


def _prep_inputs(data, mask, emb, W_ih, W_hh, b, Wr, br, bd):
    assert not np.any(b) and not np.any(br), "nonzero LSTM/readout bias unsupported"
    bf = ml_dtypes.bfloat16
    x = np.ascontiguousarray(data[:-1]).astype(np.int64).reshape(-1)
    y = np.ascontiguousarray(data[1:]).astype(np.int64).reshape(-1)

    # one global int8 scale for emb (and thus X = emb[x], EY = emb[y])
    s8 = float(np.abs(emb).max()) / 127.0
    qemb = np.clip(np.rint(emb * (1.0 / s8)), -127, 127).astype(np.int8)

    qX = qemb[x]                                  # [R, E] int8
    # xt[mc, p, k, m] = qX[mc*128 + m, k*128 + p]
    xt = np.ascontiguousarray(
        qX.reshape(MC, 128, KC, 128).transpose(0, 3, 2, 1))
    swi = float(np.abs(W_ih).max()) / 127.0
    qwih = np.clip(np.rint(W_ih * (1.0 / swi)), -127, 127).astype(np.int8)
    wih = np.ascontiguousarray(
        qwih.reshape(KC, 128, 4 * H).transpose(1, 0, 2))
    whh = np.ascontiguousarray(
        W_hh.reshape(KC, 128, 4 * H).transpose(1, 0, 2)).astype(bf)
    swr = float(np.abs(Wr).max()) / 127.0
    qwr = np.clip(np.rint(Wr * (1.0 / swr)), -127, 127).astype(np.int8)
    # wrt[p, k, e] = Wr[e, k*128 + p]
    wrt = np.ascontiguousarray(
        qwr.T.reshape(KC, 128, E).transpose(1, 0, 2))
    qEY = qemb[y]                                 # [R, E] int8
    eyt = np.ascontiguousarray(
        qEY.T.reshape(KC, 128, R).transpose(1, 0, 2))
    ident = np.eye(64, dtype=bf)
    ones = np.ones((128, 1), dtype=bf)
    scl = np.empty((128, 2), np.float32)
    scl[:, 0] = s8 * swi      # XW dequant (PSUM->SBUF copy)
    scl[:, 1] = s8 * swr      # decoder Exp scale (outt2 carries 1/swr)

    in_maps = []
    for j in range(NC):
        shard = qemb[j * VS:(j + 1) * VS]         # [VS, E] int8
        embt = np.ascontiguousarray(
            shard.T.reshape(KC, 128, VS).transpose(1, 0, 2))
        in_maps.append({
            "xtj": np.ascontiguousarray(xt[j * MCL:(j + 1) * MCL]),
            "wihj": np.ascontiguousarray(wih[j * 16:(j + 1) * 16]),
            "whhj": np.ascontiguousarray(whh[j * 16:(j + 1) * 16]),
            "wrtj": np.ascontiguousarray(wrt[j * 16:(j + 1) * 16]),
            "embtj": embt,
            "eytj": np.ascontiguousarray(eyt[:, :, j * RL:(j + 1) * RL]),
            "ident": ident, "ones128": ones, "scl": scl,
        })
    return in_maps, (y, s8 * swr)


def _combine(results, aux, mask, bd):
    y, s8 = aux
    S = np.zeros(R, np.float64)
    for j in range(NC):
        # s_out[p, mc] -> row mc*128 + p
        S += results[j]["s_out"].T.reshape(-1).astype(np.float64)
    Tt = np.concatenate(
        [results[j]["t_out"].reshape(-1) for j in range(NC)]).astype(np.float64)
    # device dots are over 8x OUT.T (ReduceScatter of 8 copies) and int8 EY
    Tt = Tt * (s8 / 8.0) + bd[y]
    m = mask[1:].reshape(-1).astype(np.float64)
    nll = np.log(S) - Tt
    loss = (nll * m).sum() / (B * B)
    return np.float32(loss)


def _run(in_maps, **kw):
    nc = _get_program()
    return run_bass_kernel_spmd(nc, in_maps, core_ids=list(range(NC)), **kw)


def kernel(data, mask, emb, W_ih, W_hh, b, Wr, br, bd):
    data = np.asarray(data)
    mask = np.asarray(mask).astype(np.float32)
    emb = np.asarray(emb).astype(np.float32)
    args = dict(data=data, mask=mask, emb=emb,
                W_ih=np.asarray(W_ih, np.float32),
                W_hh=np.asarray(W_hh, np.float32),
                b=np.asarray(b, np.float32), Wr=np.asarray(Wr, np.float32),
                br=np.asarray(br, np.float32), bd=np.asarray(bd, np.float32))
    in_maps, aux = _prep_inputs(**args)
    res = _run(in_maps)
    return _combine(res.results, aux, mask, np.asarray(bd, np.float64))


# revision 4
# speedup vs baseline: 1.5157x; 1.2750x over previous
"""Trainium2 Bass kernel for the tied-embedding LSTM LM loss — v3.

v3 = v2 + int8 wire format for the three big data tensors (embt, xt,
eyt), all quantized with one global scale s8 = max|emb|/127.  The
integer values flow through the bf16 matmuls exactly; s8 is folded into
the XW PSUM->SBUF copy (activation scale), the decoder Exp scale, and a
host-side multiply for the target dots.  Upload drops ~48 MB.


v1 replicated every input across the 8 cores (~58 MB/core, ~464 MB per
call over the axon tunnel) — host->device transfer dominated the wall
clock.  v2 uploads each unique byte once (sharded across cores,
~14.3 MB/core) and reassembles on device with collectives:

  upload (per core j):
    xtj    [8,128,KC,128]  X rows [1024j,1024j+1024)   (2 MB)
    wihj   [16,KC,4096]    W_ih partition rows 16j..   (1 MB)
    whhj   [16,KC,4096]    W_hh partition rows 16j..   (1 MB)
    wrtj   [16,KC,1024]    Wr.T partition rows 16j..   (0.25 MB)
    embtj  [128,KC,4000]   emb.T vocab shard           (8 MB)
    eytj   [128,KC,1024]   emb[y].T/8 row shard        (2 MB)

  device:
    AllGather W_ih, W_hh, Wr.T
    Phase A (row-sharded): XWj = Xj @ W_ih ; AllGather XW
    Phase B (replicated): 128-step LSTM recurrence; emits OUT.T into a
      row-block-outer DRAM layout outt2[rb, p, k, rl]
    ReduceScatter(outt2) -> or2 = 8 * OUT.T[:, rows_j]  (fixed address,
      avoids core-dependent offsets; the 8x is folded into eytj)
    Phase C: target-logit dots for local rows; vocab-sharded decoder
      sum(exp(logit)) partials
  host: combine partials -> scalar loss.
"""

import numpy as np
import ml_dtypes

import jax
jax.config.update("jax_compilation_cache_dir", "/tmp/jax_neff_cache")
jax.config.update("jax_persistent_cache_min_entry_size_bytes", -1)
jax.config.update("jax_persistent_cache_min_compile_time_secs", 0.0)

import concourse.bass as bass
import concourse.bacc as bacc
import concourse.mybir as mybir
import concourse.tile as tile
from concourse.bass_utils import run_bass_kernel_spmd

FP32 = mybir.dt.float32
BF16 = mybir.dt.bfloat16
I8 = mybir.dt.int8
AF = mybir.ActivationFunctionType
ALU = mybir.AluOpType

V, E, H = 32000, 1024, 1024
T1, B = 129, 64
TX = T1 - 1               # 128 recurrence steps
R = TX * B                # 8192 (t,b) rows
NC = 8                    # cores
VS = V // NC              # 4000 vocab shard
KC = E // 128             # 8 contraction chunks
MC = R // 128             # 64 global row chunks
MCL = MC // NC            # 8 local row chunks
RL = R // NC              # 1024 local rows
NBLK = 16                 # 512-wide OUT.T column blocks
BW = R // NBLK            # 512
RG = [list(range(NC))]


def build_program():
    nc = bacc.Bacc("TRN2", target_bir_lowering=False, num_devices=NC)

    # ---- sharded inputs (per-core layouts prepared on host) ----
    xtj = nc.dram_tensor("xtj", [MCL, 128, KC, 128], I8, kind="ExternalInput")
    wihj = nc.dram_tensor("wihj", [16, KC, 4 * H], I8, kind="ExternalInput")
    whhj = nc.dram_tensor("whhj", [16, KC, 4 * H], BF16, kind="ExternalInput")
    wrtj = nc.dram_tensor("wrtj", [16, KC, E], I8, kind="ExternalInput")
    embtj = nc.dram_tensor("embtj", [128, KC, VS], I8, kind="ExternalInput")
    eytj = nc.dram_tensor("eytj", [128, KC, RL], I8, kind="ExternalInput")
    ident = nc.dram_tensor("ident", [64, 64], BF16, kind="ExternalInput")
    ones128 = nc.dram_tensor("ones128", [128, 1], BF16, kind="ExternalInput")
    scl = nc.dram_tensor("scl", [128, 2], FP32, kind="ExternalInput")

    # ---- outputs ----
    out_all = nc.dram_tensor("out_all", [128, MC + 8], FP32, kind="ExternalOutput")

    # ---- DRAM scratch ----
    wih_b = nc.dram_tensor("wih_b", [16, KC, 4 * H], I8, kind="Internal")
    whh_b = nc.dram_tensor("whh_b", [16, KC, 4 * H], BF16, kind="Internal")
    wrt_b = nc.dram_tensor("wrt_b", [16, KC, E], I8, kind="Internal")
    wih_g = nc.dram_tensor("wih_g", [128, KC, 4 * H], I8, kind="Internal",
                           addr_space="Shared")
    whh_g = nc.dram_tensor("whh_g", [128, KC, 4 * H], BF16, kind="Internal",
                           addr_space="Shared")
    wrt_g = nc.dram_tensor("wrt_g", [128, KC, E], I8, kind="Internal",
                           addr_space="Shared")
    xwj_d = nc.dram_tensor("xwj_d", [MCL, 128, 4 * H], BF16, kind="Internal")
    xw_g = nc.dram_tensor("xw_g", [MC, 128, 4 * H], BF16, kind="Internal",
                          addr_space="Shared")
    # OUT.T in row-block-outer layout: outt2[rb, p, k, rl] =
    #   OUT.T[k*128+p, rb*1024+rl]
    outt2_d = nc.dram_tensor("outt2_d", [NC, 128, KC, RL], BF16, kind="Internal")
    or2_d = nc.dram_tensor("or2_d", [128, KC, RL], BF16, kind="Internal")

    with tile.TileContext(nc) as tc:
        with (
            tc.tile_pool(name="psum", bufs=2, space="PSUM") as pp,
            tc.tile_pool(name="small", bufs=1) as smp,
        ):
            id_sb = smp.tile([64, 64], BF16, tag="id")
            nc.sync.dma_start(id_sb[:], ident[:])
            ones_sb = smp.tile([128, 1], BF16, tag="ones")
            nc.sync.dma_start(ones_sb[:], ones128[:])
            scl_sb = smp.tile([128, 2], FP32, tag="scl")
            nc.sync.dma_start(scl_sb[:], scl[:])
            s_sb = smp.tile([128, MC + 8], FP32, tag="s")

            # ---- bounce sharded weights into internal DRAM, then gather ----
            nc.gpsimd.dma_start(wih_b[:], wihj[:])
            nc.gpsimd.collective_compute(
                "AllGather", ALU.bypass, replica_groups=RG,
                ins=[wih_b[:]], outs=[wih_g[:]])
            nc.gpsimd.dma_start(whh_b[:], whhj[:])
            nc.gpsimd.collective_compute(
                "AllGather", ALU.bypass, replica_groups=RG,
                ins=[whh_b[:]], outs=[whh_g[:]])
            nc.gpsimd.dma_start(wrt_b[:], wrtj[:])
            nc.gpsimd.collective_compute(
                "AllGather", ALU.bypass, replica_groups=RG,
                ins=[wrt_b[:]], outs=[wrt_g[:]])

            # ================= Phase A: XWj = Xj @ W_ih (row-sharded) =======
            with (
                tc.tile_pool(name="wih_p", bufs=1) as wih_p,
                tc.tile_pool(name="a_io", bufs=3) as a_io,
            ):
                wih_sb = wih_p.tile([128, KC, 4 * H], BF16, tag="w")
                with tc.tile_pool(name="astage", bufs=1) as ast_p:
                    wih_q = ast_p.tile([128, KC, 4 * H], I8, tag="wq")
                    nc.sync.dma_start(wih_q[:], wih_g[:])
                    nc.any.tensor_copy(wih_sb[:], wih_q[:])
                for mc in range(MCL):
                    xt_q = a_io.tile([128, KC, 128], I8, tag="xtq")
                    nc.sync.dma_start(xt_q[:], xtj[mc])
                    xt_sb = a_io.tile([128, KC, 128], BF16, tag="xt")
                    nc.any.tensor_copy(xt_sb[:], xt_q[:])
                    for hf in range(2):
                        ps = pp.tile([128, 2048], FP32, tag="ps")
                        for k in range(KC):
                            for nn in range(4):
                                nc.tensor.matmul(
                                    ps[:, nn * 512:(nn + 1) * 512],
                                    lhsT=xt_sb[:, k, :],
                                    rhs=wih_sb[:, k, hf * 2048 + nn * 512:
                                               hf * 2048 + (nn + 1) * 512],
                                    start=(k == 0), stop=(k == KC - 1),
                                )
                        xw_sb = a_io.tile([128, 2048], BF16, tag="xw")
                        # dequant: XW = s8 * (qX @ W_ih)
                        nc.scalar.activation(xw_sb[:], ps[:], AF.Copy,
                                             scale=scl_sb[:, 0:1])
                        nc.sync.dma_start(
                            xwj_d[mc, :, hf * 2048:(hf + 1) * 2048], xw_sb[:])

            nc.gpsimd.collective_compute(
                "AllGather", ALU.bypass, replica_groups=RG,
                ins=[xwj_d[:]], outs=[xw_g[:]])

            # ================= Phase B: LSTM recurrence (replicated) ========
            with (
                tc.tile_pool(name="whh_p", bufs=1) as whh_p,
                tc.tile_pool(name="b_io", bufs=2) as b_io,
                tc.tile_pool(name="b_st", bufs=2) as b_st,
            ):
                whh_sb = whh_p.tile([128, KC, 4 * H], BF16, tag="w")
                nc.sync.dma_start(whh_sb[:], whh_g[:])
                wrt_sb = whh_p.tile([128, KC, E], BF16, tag="wrt")
                with tc.tile_pool(name="wstage", bufs=1) as wst:
                    wrt_q = wst.tile([128, KC, E], I8, tag="wrtq")
                    nc.sync.dma_start(wrt_q[:], wrt_g[:])
                    nc.any.tensor_copy(wrt_sb[:], wrt_q[:])

                ht_sb = b_st.tile([128, KC, 64], BF16, tag="ht")
                ct_sb = b_st.tile([64, H], FP32, tag="ct")
                nc.any.memset(ht_sb[:], 0.0)
                nc.any.memset(ct_sb[:], 0.0)

                for t in range(TX):
                    xwb = b_io.tile([64, 4 * H], BF16, tag="xwb")
                    nc.sync.dma_start(
                        xwb[:], xw_g[t // 2, (t % 2) * 64:(t % 2) * 64 + 64, :])

                    ghalf = []
                    for hf in range(2):
                        g = pp.tile([64, 2048], FP32, tag="ps")
                        for nn in range(4):
                            nc.tensor.matmul(
                                g[:, nn * 512:(nn + 1) * 512],
                                lhsT=id_sb[:],
                                rhs=xwb[:, hf * 2048 + nn * 512:
                                        hf * 2048 + (nn + 1) * 512],
                                start=True, stop=False,
                            )
                        for k in range(KC):
                            for nn in range(4):
                                nc.tensor.matmul(
                                    g[:, nn * 512:(nn + 1) * 512],
                                    lhsT=ht_sb[:, k, :],
                                    rhs=whh_sb[:, k, hf * 2048 + nn * 512:
                                               hf * 2048 + (nn + 1) * 512],
                                    start=False, stop=(k == KC - 1),
                                )
                        ghalf.append(g)

                    gates = b_io.tile([64, 4 * H], FP32, tag="gates")
                    # layout: [i | f] from half0, [gg | o] from half1
                    nc.scalar.activation(gates[:, 0:2048], ghalf[0][:, 0:2048],
                                         AF.Sigmoid)
                    nc.scalar.activation(gates[:, 2048:3072], ghalf[1][:, 0:1024],
                                         AF.Tanh)
                    nc.scalar.activation(gates[:, 3072:4096], ghalf[1][:, 1024:2048],
                                         AF.Sigmoid)

                    t1 = b_io.tile([64, H], FP32, tag="t1")
                    nc.vector.tensor_tensor(t1[:], gates[:, 0:1024],
                                            gates[:, 2048:3072], op=ALU.mult)
                    t2 = b_io.tile([64, H], FP32, tag="t2")
                    nc.vector.tensor_tensor(t2[:], gates[:, 1024:2048],
                                            ct_sb[:], op=ALU.mult)
                    cn = b_st.tile([64, H], FP32, tag="ct")
                    nc.vector.tensor_tensor(cn[:], t1[:], t2[:], op=ALU.add)
                    tn = b_io.tile([64, H], FP32, tag="tn")
                    nc.scalar.activation(tn[:], cn[:], AF.Tanh)
                    hn = b_io.tile([64, H], BF16, tag="hn")
                    nc.vector.tensor_tensor(hn[:], gates[:, 3072:4096], tn[:],
                                            op=ALU.mult)
                    ct_sb = cn

                    trp = pp.tile([128, 512], BF16, tag="ps")
                    for k in range(KC):
                        nc.tensor.transpose(
                            trp[:, k * 64:(k + 1) * 64],
                            hn[:, k * 128:(k + 1) * 128], id_sb[:])
                    ht_sb = b_st.tile([128, KC, 64], BF16, tag="ht")
                    nc.any.tensor_copy(ht_sb[:], trp[:])

                    # readout OUT.T columns for this step
                    rop = pp.tile([128, 512], FP32, tag="ps")
                    for m in range(KC):
                        for k in range(KC):
                            nc.tensor.matmul(
                                rop[:, m * 64:(m + 1) * 64],
                                lhsT=wrt_sb[:, k, m * 128:(m + 1) * 128],
                                rhs=ht_sb[:, k, :],
                                start=(k == 0), stop=(k == KC - 1))
                    ro_sb = b_io.tile([128, KC, 64], BF16, tag="ro")
                    nc.any.tensor_copy(ro_sb[:], rop[:])
                    # step t covers rows t*64..t*64+63 ->
                    # rb = t//16, rl = (t%16)*64
                    nc.sync.dma_start(
                        outt2_d[t // 16, :, :, (t % 16) * 64:(t % 16) * 64 + 64],
                        ro_sb[:])

            # identical outt2 on every core -> rank j keeps 8*chunk_j
            nc.gpsimd.collective_compute(
                "ReduceScatter", ALU.add, replica_groups=RG,
                ins=[outt2_d[:]], outs=[or2_d[:]])

            # ================= Phase C: readout + decoder =================
            with (
                tc.tile_pool(name="c_w", bufs=1) as c_w,
                tc.tile_pool(name="c_io", bufs=2) as c_io,
                tc.tile_pool(name="c_sc", bufs=2) as c_sc,
            ):
                embt_sb = c_w.tile([128, KC, VS], BF16, tag="embt")
                for hf in range(2):
                    embq = c_io.tile([128, KC, 2000], I8, tag="embq")
                    nc.sync.dma_start(embq[:], embtj[:, :, hf * 2000:(hf + 1) * 2000])
                    nc.any.tensor_copy(embt_sb[:, :, hf * 2000:(hf + 1) * 2000],
                                       embq[:])

                # target-logit dots for this core's 1024 rows (note eytj is
                # pre-scaled by 1/8 to cancel the ReduceScatter over 8
                # identical copies)
                or2_sb = c_w.tile([128, KC, RL], BF16, tag="or2")
                nc.sync.dma_start(or2_sb[:], or2_d[:])
                ey_q = c_io.tile([128, KC, RL], I8, tag="eyq")
                nc.sync.dma_start(ey_q[:], eytj[:])
                ey_sb = c_w.tile([128, KC, RL], BF16, tag="ey")
                nc.any.tensor_copy(ey_sb[:], ey_q[:])
                prod = c_w.tile([128, KC, RL], BF16, tag="prod")
                nc.vector.tensor_tensor(prod[:], or2_sb[:], ey_sb[:], op=ALU.mult)
                # t2[p, c] = t[c*128 + p]: reduce over e via rhs=ones
                tps = pp.tile([128, 8], FP32, tag="ps")
                for c in range(8):
                    for k in range(KC):
                        nc.tensor.matmul(
                            tps[:, c:c + 1],
                            lhsT=prod[:, k, c * 128:(c + 1) * 128],
                            rhs=ones_sb[:],
                            start=(k == 0), stop=(k == KC - 1))
                nc.any.tensor_copy(s_sb[:, MC:MC + 8], tps[:])

                # decoder sum(exp(logit)) partials over the vocab shard
                for nb in range(NBLK):
                    outt = c_io.tile([128, KC, BW], BF16, tag="outt")
                    nc.sync.dma_start(
                        outt[:],
                        outt2_d[nb // 2, :, :,
                                (nb % 2) * BW:(nb % 2) * BW + BW])

                    for mm in range(4):
                        gmc = nb * 4 + mm
                        sacc = c_sc.tile([128, 2], FP32, tag="sacc")
                        for hf in range(2):
                            ps2 = pp.tile([128, 2000], FP32, tag="ps")
                            for k in range(KC):
                                for nn in range(4):
                                    nc.tensor.matmul(
                                        ps2[:, nn * 500:(nn + 1) * 500],
                                        lhsT=outt[:, k, mm * 128:(mm + 1) * 128],
                                        rhs=embt_sb[:, k, hf * 2000 + nn * 500:
                                                    hf * 2000 + (nn + 1) * 500],
                                        start=(k == 0), stop=(k == KC - 1))
                            esc = c_sc.tile([128, 2000], BF16, tag="esc")
                            # logit = s8 * (out . qemb)
                            nc.scalar.activation(esc[:], ps2[:], AF.Exp,
                                                 scale=scl_sb[:, 1:2],
                                                 accum_out=sacc[:, hf:hf + 1])
                        nc.vector.tensor_tensor(s_sb[:, gmc:gmc + 1],
                                                sacc[:, 0:1], sacc[:, 1:2],
                                                op=ALU.add)

            nc.sync.dma_start(out_all[:], s_sb[:])

    nc.compile()
    return nc


_PROGRAM = None


def _get_program():
    global _PROGRAM
    if _PROGRAM is None:
        _PROGRAM = build_program()
    return _PROGRAM


def _prep_inputs(data, mask, emb, W_ih, W_hh, b, Wr, br, bd):
    assert not np.any(b) and not np.any(br), "nonzero LSTM/readout bias unsupported"
    bf = ml_dtypes.bfloat16
    x = np.ascontiguousarray(data[:-1]).astype(np.int64).reshape(-1)
    y = np.ascontiguousarray(data[1:]).astype(np.int64).reshape(-1)

    # one global int8 scale for emb (and thus X = emb[x], EY = emb[y])
    s8 = float(np.abs(emb).max()) / 127.0
    qemb = np.clip(np.rint(emb * (1.0 / s8)), -127, 127).astype(np.int8)

    qX = qemb[x]                                  # [R, E] int8
    # xt[mc, p, k, m] = qX[mc*128 + m, k*128 + p]
    xt = np.ascontiguousarray(
        qX.reshape(MC, 128, KC, 128).transpose(0, 3, 2, 1))
    swi = float(np.abs(W_ih).max()) / 127.0
    qwih = np.clip(np.rint(W_ih * (1.0 / swi)), -127, 127).astype(np.int8)
    wih = np.ascontiguousarray(
        qwih.reshape(KC, 128, 4 * H).transpose(1, 0, 2))
    whh = np.ascontiguousarray(
        W_hh.reshape(KC, 128, 4 * H).transpose(1, 0, 2)).astype(bf)
    swr = float(np.abs(Wr).max()) / 127.0
    qwr = np.clip(np.rint(Wr * (1.0 / swr)), -127, 127).astype(np.int8)
    # wrt[p, k, e] = Wr[e, k*128 + p]
    wrt = np.ascontiguousarray(
        qwr.T.reshape(KC, 128, E).transpose(1, 0, 2))
    qEY = qemb[y]                                 # [R, E] int8
    eyt = np.ascontiguousarray(
        qEY.T.reshape(KC, 128, R).transpose(1, 0, 2))
    ident = np.eye(64, dtype=bf)
    ones = np.ones((128, 1), dtype=bf)
    scl = np.empty((128, 2), np.float32)
    scl[:, 0] = s8 * swi      # XW dequant (PSUM->SBUF copy)
    scl[:, 1] = s8 * swr      # decoder Exp scale (outt2 carries 1/swr)

    in_maps = []
    for j in range(NC):
        shard = qemb[j * VS:(j + 1) * VS]         # [VS, E] int8
        embt = np.ascontiguousarray(
            shard.T.reshape(KC, 128, VS).transpose(1, 0, 2))
        in_maps.append({
            "xtj": np.ascontiguousarray(xt[j * MCL:(j + 1) * MCL]),
            "wihj": np.ascontiguousarray(wih[j * 16:(j + 1) * 16]),
            "whhj": np.ascontiguousarray(whh[j * 16:(j + 1) * 16]),
            "wrtj": np.ascontiguousarray(wrt[j * 16:(j + 1) * 16]),
            "embtj": embt,
            "eytj": np.ascontiguousarray(eyt[:, :, j * RL:(j + 1) * RL]),
            "ident": ident, "ones128": ones, "scl": scl,
        })
    return in_maps, (y, s8 * swr)


def _combine(results, aux, mask, bd):
    y, s8 = aux
    S = np.zeros(R, np.float64)
    for j in range(NC):
        # out_all[p, mc] -> row mc*128 + p
        S += results[j]["out_all"][:, :MC].T.reshape(-1).astype(np.float64)
    # out_all[p, MC + c] -> local row c*128 + p
    Tt = np.concatenate(
        [results[j]["out_all"][:, MC:].T.reshape(-1)
         for j in range(NC)]).astype(np.float64)
    # device dots are over 8x OUT.T (ReduceScatter of 8 copies) and int8 EY
    Tt = Tt * (s8 / 8.0) + bd[y]
    m = mask[1:].reshape(-1).astype(np.float64)
    nll = np.log(S) - Tt
    loss = (nll * m).sum() / (B * B)
    return np.float32(loss)


def _run(in_maps, **kw):
    nc = _get_program()
    return run_bass_kernel_spmd(nc, in_maps, core_ids=list(range(NC)), **kw)


def kernel(data, mask, emb, W_ih, W_hh, b, Wr, br, bd):
    data = np.asarray(data)
    mask = np.asarray(mask).astype(np.float32)
    emb = np.asarray(emb).astype(np.float32)
    args = dict(data=data, mask=mask, emb=emb,
                W_ih=np.asarray(W_ih, np.float32),
                W_hh=np.asarray(W_hh, np.float32),
                b=np.asarray(b, np.float32), Wr=np.asarray(Wr, np.float32),
                br=np.asarray(br, np.float32), bd=np.asarray(bd, np.float32))
    in_maps, aux = _prep_inputs(**args)
    res = _run(in_maps)
    return _combine(res.results, aux, mask, np.asarray(bd, np.float64))
